# revision 32
# baseline (speedup 1.0000x reference)
"""Trainium2 Bass kernel for nn_KernelAxialMultiAttention (linear attention).

Math (per independent (b, m) slice; x: [T=256, C=512], N=8 heads, D=64):
  q = elu(x @ Wq.T) + 1          [T, C]   (heads along C)
  k = elu(x @ Wk.T) + 1
  ksum[c]   = sum_t k[t, c]
  krow[t,n] = sum_{c in head n} k[c, t]
  zden[n,t] = sum_{c in head n} q[c, t] * ksum[c];  z = 1/zden
  s[c, n]   = sum_t x[t, c] * krow[t, n]
  u[n, e]   = sum_c s[c, n] * Wv[n*D+e, c]     (= KtV column sums)
  w[n, cO]  = sum_e u[n, e] * Wp[cO, n*D+e]
  out[t,cO] = sum_n z[n, t] * w[n, cO]
Algebraically identical to the reference (sum reordering only; Z constant
over D collapses the v/out projections).

elu(x)+1 is computed as min(exp(x), relu(x) + 1)  (exact identity).

Engine mapping: all matmuls stream bf16 (1 cyc/row); the nonlinearity is
split ACT (exp, half the relus) / DVE (fused min-combine, ksum via
tensor_reduce); the masked stationaries (wz, gm) are built with small
strided copies instead of per-partition-scalar ops; nothing hot runs on
GpSimd. Emission is software-pipelined: pair p's small matmuls sit behind
pair p+1's projection matmuls in the PE queue.

Sharding: data-parallel over the 128 (b, m) slices -> 16 per NeuronCore.
"""

import os
import sys

import numpy as np

for _p in ("/opt/trn_rl_repo", "/root/.axon_site/_ro/trn_rl_repo"):
    if os.path.isdir(_p) and _p not in sys.path:
        sys.path.insert(0, _p)

B, M, T, C = 2, 64, 256, 512
NH, D = 8, 64
S = 16            # slices per core
S2 = S // 2       # pairs per core
NCORES = 8
P = 128           # partitions
NKC = C // P      # 4 c chunks
NTC = T // P      # 2 t chunks
T2 = 2 * T        # free size covering a pair of slices
GS = 4            # slices per output group
NG = S // GS      # 2 groups

_BUILT = {}


def _build_nc():
    from contextlib import ExitStack

    import concourse.bacc as bacc
    import concourse.bass as bass
    import concourse.mybir as mybir
    import concourse.tile as tile

    f32 = mybir.dt.float32
    bf16 = mybir.dt.bfloat16
    fp8 = mybir.dt.float8e4
    AF = mybir.ActivationFunctionType
    OP = mybir.AluOpType
    AX = mybir.AxisListType

    nc = bacc.Bacc(None, target_bir_lowering=False)
    # xn: (pair, part=t%128, si, tcb, c) ; xT: (pair, part=c%128, kc, si, t)
    xn_d = nc.declare_dram_parameter("xn", [S2, P, 2, NTC, C], bf16, isOutput=False)
    xT_d = nc.declare_dram_parameter("xT", [S2, P, 2, 2, 2, T], fp8, isOutput=False)
    wqT_d = nc.declare_dram_parameter("WqT", [C, C], fp8, isOutput=False)
    wkT_d = nc.declare_dram_parameter("WkT", [C, C], fp8, isOutput=False)
    wvT_d = nc.declare_dram_parameter("WvT", [C, C], bf16, isOutput=False)
    wpT_d = nc.declare_dram_parameter("WpT", [C, C], bf16, isOutput=False)
    out_d = nc.declare_dram_parameter("out", [S, P, NTC, C], f32, isOutput=True)
    dump = os.environ.get("KERNEL_DUMP", "0") == "1"
    if dump:
        dbg_qe = nc.declare_dram_parameter("dbg_qe", [P, NKC, T2], bf16, isOutput=True)
        dbg_ke = nc.declare_dram_parameter("dbg_ke", [P, NKC, 2, T], bf16, isOutput=True)
        dbg_ksum = nc.declare_dram_parameter("dbg_ksum", [P, NKC, 2], f32, isOutput=True)
        dbg_krt = nc.declare_dram_parameter("dbg_krt", [P, 2, NTC, NH], bf16, isOutput=True)
        dbg_sT = nc.declare_dram_parameter("dbg_sT", [P, NKC, S, NH], bf16, isOutput=True)
        dbg_z = nc.declare_dram_parameter("dbg_z", [P, S2, T2], bf16, isOutput=True)
        dbg_ut = nc.declare_dram_parameter("dbg_ut", [P, NKC, GS], bf16, isOutput=True)
        dbg_gm = nc.declare_dram_parameter("dbg_gm", [P, NKC, 2, 64], bf16, isOutput=True)
        dbg_w4 = nc.declare_dram_parameter("dbg_w4", [P, C], bf16, isOutput=True)

    with tile.TileContext(nc) as tc, ExitStack() as ctx:
        wpool = ctx.enter_context(tc.tile_pool(name="weights", bufs=1))
        persist = ctx.enter_context(tc.tile_pool(name="persist", bufs=1))
        xn_pool = ctx.enter_context(tc.tile_pool(name="xn", bufs=4))
        xt_pool = ctx.enter_context(tc.tile_pool(name="xT", bufs=4))
        qe_pool = ctx.enter_context(tc.tile_pool(name="qe", bufs=3))
        ke_pool = ctx.enter_context(tc.tile_pool(name="ke", bufs=3))
        e_pool = ctx.enter_context(tc.tile_pool(name="etile", bufs=3))
        r_pool = ctx.enter_context(tc.tile_pool(name="rtile", bufs=3))
        ksum_pool = ctx.enter_context(tc.tile_pool(name="ksum", bufs=3))
        wz_pool = ctx.enter_context(tc.tile_pool(name="wz", bufs=2))
        krt_pool = ctx.enter_context(tc.tile_pool(name="krt", bufs=2))
        zf_pool = ctx.enter_context(tc.tile_pool(name="zf", bufs=2))
        ut_pool = ctx.enter_context(tc.tile_pool(name="ut", bufs=2))
        gm_pool = ctx.enter_context(tc.tile_pool(name="gm", bufs=2))
        w4_pool = ctx.enter_context(tc.tile_pool(name="w4", bufs=2))
        osb_pool = ctx.enter_context(tc.tile_pool(name="outsb", bufs=3))

        ps_proj = ctx.enter_context(
            tc.tile_pool(name="ps_proj", bufs=2, space=bass.MemorySpace.PSUM))
        ps_small = ctx.enter_context(
            tc.tile_pool(name="ps_small", bufs=2, space=bass.MemorySpace.PSUM))
        ps_z = ctx.enter_context(
            tc.tile_pool(name="ps_z", bufs=2, space=bass.MemorySpace.PSUM))
        ps_out = ctx.enter_context(
            tc.tile_pool(name="ps_out", bufs=2, space=bass.MemorySpace.PSUM))

        # ---- weights (host-pretransposed [c_in, c_out]) into SBUF ----
        wq = wpool.tile([P, 2, 2, C], fp8, tag="wq")
        wk = wpool.tile([P, 2, 2, C], fp8, tag="wk")
        wv = wpool.tile([P, NKC, C], bf16, tag="wv")
        wp = wpool.tile([P, NKC, C], bf16, tag="wp")
        for wT, wd in ((wv, wvT_d), (wp, wpT_d)):
            nc.sync.dma_start(out=wT[:], in_=wd.rearrange("(a p) d -> p a d", p=P))
        for wT, wd in ((wq, wqT_d), (wk, wkT_d)):
            nc.sync.dma_start(
                out=wT[:], in_=wd.rearrange("(b j p) d -> p b j d", p=P, j=2))

        # ---- head-block masks: maskT[p, ci, n] = 1 if head(ci*128+p) == n ----
        maskT = wpool.tile([P, NKC, NH], bf16, tag="maskT")
        nc.gpsimd.memset(maskT[:], 0.0)
        for ci in range(NKC):
            nc.gpsimd.memset(maskT[0:64, ci, 2 * ci:2 * ci + 1], 1.0)
            nc.gpsimd.memset(maskT[64:128, ci, 2 * ci + 1:2 * ci + 2], 1.0)

        sT_all = persist.tile([P, NKC, S, NH], bf16, tag="sT_all")
        # z for slice s lives at partitions 64*(s%2) + n of pair s//2
        z_all = persist.tile([P, S2, T2], bf16, tag="z_all")

        dbg_keep = {}
        xn_t = [None] * S2
        xt_t = [None] * S2
        qe_t = [None] * S2
        ke_t = [None] * S2
        ksum_t = [None] * S2
        krt_t = [None] * S2
        ew_parity = [0]

        def emit_load(p):
            xn_ = xn_pool.tile([P, 2, NTC, C], bf16, tag="xn")
            nc.sync.dma_start(out=xn_[:], in_=xn_d[p])
            xt_ = xt_pool.tile([P, 2, 2, 2, T], fp8, tag="xT")
            nc.sync.dma_start(out=xt_[:], in_=xT_d[p])
            xn_t[p], xt_t[p] = xn_, xt_

        def proj_units(p):
            """q/k projection chunks for pair p (8 units: k then q)."""
            st_ = {}
            DS = 1.0 / 64.0   # fp8 weights are scaled by 64

            def one(is_k, mc):
                if "qe" not in st_:
                    st_["qe"] = qe_pool.tile([P, NKC, T2], bf16,
                                             name="qe", tag="qe")
                    st_["ke"] = ke_pool.tile([P, NKC, 2, T], bf16,
                                             name="ke", tag="ke")
                    st_["ksum"] = ksum_pool.tile([P, NKC, 2], f32,
                                                 name="ksum", tag="ksum")
                    qe_t[p], ke_t[p], ksum_t[p] = (
                        st_["qe"], st_["ke"], st_["ksum"])
                qe, ke, ksum = st_["qe"], st_["ke"], st_["ksum"]
                wT = wk if is_k else wq
                pp = ps_proj.tile([P, T2], f32, name="pp", tag="proj")
                for kcp in range(2):
                    nc.tensor.matmul(
                        pp[:],
                        wT[:, kcp, :, mc * P:(mc + 1) * P],
                        xt_t[p][:, kcp, :, :, :],
                        start=(kcp == 0),
                        stop=(kcp == 1),
                        perf_mode=mybir.MatmulPerfMode.DoubleRow,
                    )
                # elu(x)+1 = min(exp(x), relu(x)+1), x = pp/64
                et = e_pool.tile([P, T2], bf16, name="et", tag="etile")
                nc.scalar.activation(et[:], pp[:], AF.Exp, scale=DS)
                tgt = ke[:, mc, :, :] if is_k else qe[:, mc, :]
                rt = r_pool.tile([P, T2], bf16, name="rt", tag="rtile")
                if ew_parity[0] % 2 == 0:
                    nc.scalar.activation(rt[:], pp[:], AF.Relu, scale=DS)
                else:
                    nc.vector.tensor_scalar(
                        rt[:], pp[:], DS, 0.0, OP.mult, OP.max)
                nc.vector.scalar_tensor_tensor(
                    tgt, rt[:], 1.0, et[:], OP.add, OP.min)
                ew_parity[0] += 1
                if is_k:
                    nc.vector.tensor_reduce(
                        ksum[:, mc, :], ke[:, mc, :, :], AX.X, OP.add)

            units = []
            for is_k in (True, False):
                for mc in range(NKC):
                    units.append(
                        lambda is_k=is_k, mc=mc: one(is_k, mc))
            return units

        def emit_proj(p):
            for u in proj_units(p):
                u()

        def krow_units(p):
            st_ = {}

            def one(si):
                if si == 0:
                    st_["krt_ps"] = ps_small.tile(
                        [P, 2, NTC, NH], f32, name="krt_ps", tag="sm")
                    emit_wz(p)
                ke = ke_t[p]
                for tcb in range(NTC):
                    for mc in range(NKC):
                        nc.tensor.matmul(
                            st_["krt_ps"][:, si, tcb, :],
                            ke[:, mc, si, tcb * P:(tcb + 1) * P],
                            maskT[:, mc, :],
                            start=(mc == 0),
                            stop=(mc == NKC - 1),
                        )
                if si == 1:
                    krt = krt_pool.tile([P, 2, NTC, NH], bf16,
                                        name="krt", tag="krt")
                    nc.vector.tensor_copy(krt[:], st_["krt_ps"][:])
                    krt_t[p] = krt
            return [lambda: one(0), lambda: one(1)]

        wz_t = [None] * S2

        def emit_wz(p):
            # wz[c, mc, si, n'] with n' = n in a 64-wide block: the zden
            # matmul's 128 stationary cols land slice si's heads at out
            # partitions 64*si + n.
            wz = wz_pool.tile([P, NKC, 2, 64], bf16, name="wz", tag="wz")
            nc.gpsimd.memset(wz[:], 0.0)
            ksum = ksum_t[p]
            for mc in range(NKC):
                nc.gpsimd.tensor_copy(
                    wz[0:64, mc, :, 2 * mc], ksum[0:64, mc, :])
                nc.gpsimd.tensor_copy(
                    wz[64:128, mc, :, 2 * mc + 1], ksum[64:128, mc, :])
            wz_t[p] = wz

        def st_units(p):
            st_ = {}

            def one(si):
                if si == 0:
                    st_["ps"] = ps_small.tile(
                        [P, 2, NKC, NH], f32, name="st_ps", tag="sm")
                for mc in range(NKC):
                    for tcb in range(NTC):
                        nc.tensor.matmul(
                            st_["ps"][:, si, mc, :],
                            xn_t[p][:, si, tcb, mc * P:(mc + 1) * P],
                            krt_t[p][:, si, tcb, :],
                            start=(tcb == 0),
                            stop=(tcb == NTC - 1),
                        )
                nc.vector.tensor_copy(
                    sT_all[:, :, 2 * p + si, :], st_["ps"][:, si, :, :])
            return [lambda: one(0), lambda: one(1)]

        def zden_unit(p):
            def one():
                zd_ps = ps_z.tile([P, T2], f32, name="zd_ps", tag="smz")
                for mc in range(NKC):
                    nc.tensor.matmul(
                        zd_ps[:],
                        wz_t[p][:, mc, :, :],
                        qe_t[p][:, mc, :],
                        start=(mc == 0),
                        stop=(mc == NKC - 1),
                    )
                zf = zf_pool.tile([P, T2], f32, name="zf", tag="zf")
                nc.vector.reciprocal_approx_fast(zf[:], zd_ps[:])
                nc.scalar.copy(z_all[:, p, :], zf[:])
            return [one]

        def bc_units(g):
            st_ = {}

            def ut_unit():
                ut_ps = ps_small.tile([P, NKC, GS], f32, name="ut_ps", tag="sm")
                for n in range(NH):
                    r0 = 64 * (n % 2)
                    for kc in range(NKC):
                        nc.tensor.matmul(
                            ut_ps[r0:r0 + 64, n // 2, :],
                            wv[:, kc, n * D:(n + 1) * D],
                            sT_all[:, kc, g * GS:(g + 1) * GS, n],
                            start=(kc == 0),
                            stop=(kc == NKC - 1),
                        )
                ut = ut_pool.tile([P, NKC, GS], bf16, name="ut", tag="ut")
                nc.vector.tensor_copy(ut[:], ut_ps[:])
                st_["ut"] = ut

            def w_unit(prl):
                ut = st_["ut"]
                # gm[c, ci, si, n'] masked-u stationary; 128 cols land
                # slice si's heads at out partitions 64*si + n.
                gm = gm_pool.tile([P, NKC, 2, 64], bf16, name="gm", tag="gm")
                nc.gpsimd.memset(gm[:], 0.0)
                for ci in range(NKC):
                    nc.gpsimd.tensor_copy(
                        gm[0:64, ci, :, 2 * ci],
                        ut[0:64, ci, 2 * prl:2 * prl + 2])
                    nc.gpsimd.tensor_copy(
                        gm[64:128, ci, :, 2 * ci + 1],
                        ut[64:128, ci, 2 * prl:2 * prl + 2])
                w_ps = ps_z.tile([P, C], f32, name="w_ps", tag="smz")
                for ci in range(NKC):
                    nc.tensor.matmul(
                        w_ps[:],
                        gm[:, ci, :, :],
                        wp[:, ci, :],
                        start=(ci == 0),
                        stop=(ci == NKC - 1),
                    )
                w4 = w4_pool.tile([P, C], bf16, name="w4", tag="w4")
                if prl % 2 == 0:
                    nc.scalar.copy(w4[:], w_ps[:])
                else:
                    nc.vector.tensor_copy(w4[:], w_ps[:])
                st_["w4"] = w4

            def out_unit(prl):
                pr = 2 * g + prl
                w4 = st_["w4"]
                for si in range(2):
                    osb = osb_pool.tile([P, NTC, C], f32,
                                        name="osb", tag="outsb")
                    for tcb in range(NTC):
                        o_ps = ps_out.tile([P, C], f32, name="o_ps", tag="o_ps")
                        nc.tensor.matmul(
                            o_ps[:],
                            z_all[64 * si:64 * si + 8, pr,
                                  si * T + tcb * P: si * T + (tcb + 1) * P],
                            w4[64 * si:64 * si + 8, :],
                            start=True,
                            stop=True,
                        )
                        nc.scalar.copy(osb[:, tcb, :], o_ps[:])
                    nc.sync.dma_start(out=out_d[2 * pr + si], in_=osb[:])

            return [ut_unit,
                    lambda: w_unit(0), lambda: out_unit(0),
                    lambda: w_unit(1), lambda: out_unit(1)]

        def interleave(a_units, b_units):
            n = max(len(a_units), len(b_units))
            for i in range(n):
                if i < len(a_units):
                    a_units[i]()
                if i < len(b_units):
                    b_units[i]()

        # =================== pipelined emission ===================
        # Wave p emits: proj(p+2) chunks interleaved with krow(p+1),
        # st(p), zden(p), and (even waves) the output chain of group
        # (p-2)//2 -- all fillers depend only on >=1-wave-old data, so
        # the PE queue always has ready work between projection chunks.
        emit_load(0)
        emit_load(1)
        emit_load(2)
        emit_proj(0)
        interleave(proj_units(1), krow_units(0))
        for p in range(S2):
            if p + 3 < S2:
                emit_load(p + 3)
            filler = []
            if p + 1 < S2:
                filler += krow_units(p + 1)
            filler += st_units(p)
            filler += zden_unit(p)
            if p >= 2 and p % 2 == 0:
                filler = bc_units(p // 2 - 1) + filler
            if p == S2 - 1:
                filler += bc_units(NG - 1)
            pu = proj_units(p + 2) if p + 2 < S2 else []
            interleave(pu, filler)

        if dump:
            nc.sync.dma_start(out=dbg_qe[:], in_=qe_t[7][:])
            nc.sync.dma_start(out=dbg_ke[:], in_=ke_t[7][:])
            nc.sync.dma_start(out=dbg_ksum[:], in_=ksum_t[7][:])
            nc.sync.dma_start(out=dbg_krt[:], in_=krt_t[7][:])
            nc.sync.dma_start(out=dbg_sT[:], in_=sT_all[:])
            nc.sync.dma_start(out=dbg_z[:], in_=z_all[:])

    nc.compile()
    return nc


def _get_nc():
    if "nc" not in _BUILT:
        _BUILT["nc"] = _build_nc()
    return _BUILT["nc"]


def kernel(**inputs):
    import ml_dtypes

    bf16 = ml_dtypes.bfloat16
    x = np.asarray(inputs["x"], dtype=np.float32)
    Wq = np.asarray(inputs["Wq"], dtype=np.float32)
    Wk = np.asarray(inputs["Wk"], dtype=np.float32)
    Wv = np.asarray(inputs["Wv"], dtype=np.float32)
    Wp = np.asarray(inputs["Wp"], dtype=np.float32)
    bp = np.asarray(inputs.get("bp", np.zeros(C)), dtype=np.float32)

    fp8 = ml_dtypes.float8_e4m3
    xs = x.reshape(B * M, T, C)
    wqT = np.ascontiguousarray((Wq.T * 64.0).astype(fp8))
    wkT = np.ascontiguousarray((Wk.T * 64.0).astype(fp8))
    wvT = np.ascontiguousarray(Wv.T.astype(bf16))
    wpT = np.ascontiguousarray(Wp.T.astype(bf16))

    in_maps = []
    for i in range(NCORES):
        sl = xs[S * i:S * (i + 1)]                        # [16, 256, 512] f32
        # xn: (pair, part=t%128, si, tcb, c) bf16
        xn_h = (sl.astype(bf16)
                .reshape(S2, 2, NTC, P, C).transpose(0, 3, 1, 2, 4))
        # xT: (pair, part=c%128, kcp, j, si, t) fp8, c = (2*kcp+j)*128+part
        xT_h = (sl.transpose(0, 2, 1)                     # [16, 512, 256]
                .reshape(S2, 2, 2, 2, P, T).transpose(0, 4, 2, 3, 1, 5)
                .astype(fp8))
        in_maps.append({
            "xn": np.ascontiguousarray(xn_h),
            "xT": np.ascontiguousarray(xT_h),
            "WqT": wqT, "WkT": wkT, "WvT": wvT, "WpT": wpT,
        })

    from concourse.bass_utils import run_bass_kernel_spmd

    nc = _get_nc()
    trace = os.environ.get("KERNEL_TRACE", "0") == "1"
    res = run_bass_kernel_spmd(nc, in_maps, list(range(NCORES)), trace=trace)
    if trace and res.exec_time_ns is not None:
        print(f"HW exec time: {res.exec_time_ns} ns", flush=True)
        _BUILT["exec_time_ns"] = res.exec_time_ns

    out = np.concatenate(
        [res.results[i]["out"].transpose(0, 2, 1, 3).reshape(S, T, C)
         for i in range(NCORES)], axis=0)
    out = out.reshape(B, M, T, C)
    if np.any(bp):
        out = out + bp
    return out.astype(np.float32)
